# revision 7
# baseline (speedup 1.0000x reference)
"""Distillation loss (KL + CE) kernel for Trainium2, 8 NeuronCores.

v2 strategy (vocab-major / transposed layout, PE-based reductions):
  - Flatten logits to [N=4096, V=32000]; shard 512 rows per core; cast to
    fp16 on the host.  Host also TRANSPOSES each core's slice to
    vocab-major [V, 512] and retiles it to [25 groups, 128 vocab
    partitions, 10 vocab-chunks x 512 rows] so each SBUF tile is one
    contiguous 1.31 MB DMA.
  - Per core the engines split the work (per-pass numbers, 16.4M elems):
      ACT (the hard floor, ~220us): es = exp(s/4), et = exp(t/4).
      DVE (3 fp16 2x TT passes, ~205us): d = t - s, sq = es*es, b = sq*sq.
      PE  (previously idle): all reductions, accumulated in fp32 PSUM
          across the full vocab:
            W-diag:  out[m,n] += sum_v et[v,m] * d[v,n]   per 128-row
                     block (the diagonal is sum_v et*d = the KL cross
                     term; host extracts it).  Stationary = et block.
            A/C/B:   ones-stationary column reductions: out[1, 512] +=
                     sum_v x[v, r] for x in {es, et, b}.
  - No max-subtraction (randn inputs; exp stays in fp16/fp32 range).
  - Host (float64) combine:
       KL_row  = W / (T*C) + ln A - ln C
       distill = T^2 * mean(KL_row)
       nll_row = ln B - s[row, label]   (label gather on host, f32 exact)
       task    = sum(nll*valid) / max(sum(valid), 1), valid = label != 0
       total   = alpha*distill + (1-alpha)*task
"""

import numpy as np

import concourse.bass as bass
import concourse.mybir as mybir
from concourse import tile
from concourse.bass_utils import run_bass_kernel_spmd
from concourse.vector_clock import ScopedClock, VectorClock


# ---------------------------------------------------------------------------
# Workaround: the walrus build in this image rejects instructions that carry
# more than one sync wait ("Too many sync wait commands", setupSyncWait).
# Tile freely assigns several waits to one instruction.  Two patches:
#   1. _lower_ordered_insts: before lowering, hoist excess waits from every
#      scheduled instruction onto same-engine NoOps inserted just before it.
#   2. _drain_and_barrier: the kernel-tail drain gets the whole global
#      vector clock on one instruction; emit one drain per logical proc.
# ---------------------------------------------------------------------------
_MAX_WAITS = 1


def _split_inst_waits(nc, ordered):
    for bb_name, insts in ordered.items():
        out = []
        for inst in insts:
            si = inst.sync_info
            if si is not None and si.on_wait and len(si.on_wait) > _MAX_WAITS:
                waits = list(si.on_wait)
                excess, keep = waits[:-_MAX_WAITS], waits[-_MAX_WAITS:]
                for i in range(0, len(excess), _MAX_WAITS):
                    nop = mybir.InstNoOp(
                        name=nc.get_next_instruction_name(),
                        engine=inst.engine,
                        sync_info=mybir.SyncInfo(
                            on_wait=excess[i : i + _MAX_WAITS], on_update=[]
                        ),
                    )
                    out.append(nop)
                inst.sync_info = mybir.SyncInfo(
                    on_wait=keep, on_update=list(si.on_update)
                )
            out.append(inst)
        ordered[bb_name] = out


_orig_lower_ordered_insts = tile.TileContext._lower_ordered_insts


def _patched_lower_ordered_insts(self, ordered):
    _split_inst_waits(self.nc, ordered)
    return _orig_lower_ordered_insts(self, ordered)


def _split_drain_and_barrier(self, tick_clock, wait_clock):
    nc = self.nc
    gc = tick_clock.global_clock
    n = len(gc)
    for p in range(n):
        t = gc[p]
        if t <= 0:
            continue
        vec = [0] * n
        vec[p] = t
        di = nc.sync.drain()
        wait_clock.add_sem_waits(di.ins, ScopedClock({None: VectorClock(vec)}))
    nc.all_engine_barrier()
    assert self.sems is not None
    popped = nc._tile_sem_poison_stack.pop()
    assert popped is self._sem_poison
    nc.clear_and_free_semaphores(list(self.sems.allocated().values()))
    nc.all_engine_barrier()


if not getattr(tile.TileContext, "_dloss_patched", False):
    tile.TileContext._lower_ordered_insts = _patched_lower_ordered_insts
    tile.TileContext._drain_and_barrier = _split_drain_and_barrier
    tile.TileContext._dloss_patched = True

# ---------------------------------------------------------------------------

# Problem constants (hardcoded per spec nn_DistillationLoss_52982716564146)
B, S, V = 4, 1024, 32000
N = B * S                      # 4096 rows
N_CORES = 8
R = N // N_CORES               # 512 rows per core
P = 128                        # SBUF partitions
RB = R // P                    # 4 row-blocks per core
VCH = P                        # vocab per chunk (partition dim)
N_VCH = V // VCH               # 250 vocab-chunks
G_VCH = 20                     # vocab-chunks per group
GROUP_SIZES = [G_VCH] * (N_VCH // G_VCH) + (
    [N_VCH % G_VCH] if N_VCH % G_VCH else []
)                              # 12 x 20 + 1 x 10
N_G = len(GROUP_SIZES)
GW = G_VCH * R                 # max group tile free width = 10240
TEMP = 4.0
ALPHA = 0.7
IGNORE_INDEX = 0

FP32 = mybir.dt.float32
FP16 = mybir.dt.float16
EXP = mybir.ActivationFunctionType.Exp
MULT = mybir.AluOpType.mult
SUB = mybir.AluOpType.subtract

TRACE = False
LAST_RESULT = None


def build_program():
    """Build the SPMD Bass program (identical on all cores).

    Inputs  (per core): t_vm, s_vm [N_G, 128, GW] fp16 (vocab-major tiles,
        last group zero-padded).
    Outputs (per core): o_w [128, R] f32   (diag holds W = sum et*(t-s)),
                        o_acb [65, R] f32 (rows 0/32/64 = A, C, B sums).

    The exp activations run in place (et overwrites t, es overwrites s) so
    larger group tiles fit in SBUF (fewer ACT instructions, less per-
    instruction overhead).  The A/C/B ones-stationary reductions are issued
    to distinct 32-column PE groups (tile_position (0,0)/(0,32)/(0,64)) so
    the hardware runs all three concurrently on separate XBUSes.
    """
    nc = bass.Bass(
        "TRN2",
        target_bir_lowering=False,
        debug=False,
        num_devices=N_CORES,
    )
    t_in = nc.dram_tensor("t_vm", [N_G, P, GW], FP16, kind="ExternalInput")
    s_in = nc.dram_tensor("s_vm", [N_G, P, GW], FP16, kind="ExternalInput")
    o_w = nc.dram_tensor("o_w", [P, R], FP32, kind="ExternalOutput")
    o_acb = nc.dram_tensor("o_acb", [65, R], FP32, kind="ExternalOutput")

    with tile.TileContext(nc) as tc:
        with (
            tc.tile_pool(name="t_pool", bufs=2) as t_pool,
            tc.tile_pool(name="s_pool", bufs=2) as s_pool,
            tc.tile_pool(name="d_pool", bufs=2) as d_pool,
            tc.tile_pool(name="sq_pool", bufs=2) as sq_pool,
            tc.tile_pool(name="b_pool", bufs=2) as b_pool,
            tc.tile_pool(name="const", bufs=1) as const_pool,
            tc.tile_pool(name="psum", bufs=1, space="PSUM") as psum_pool,
        ):
            ones = const_pool.tile([P, 1], FP16, tag="ones")
            nc.gpsimd.memset(ones[:], 1.0)

            ps_w = psum_pool.tile([P, R], FP32, tag="ps_w")
            ps_acb = psum_pool.tile([65, R], FP32, tag="ps_acb")

            for g, gvc in enumerate(GROUP_SIZES):
                gw = gvc * R
                t_t = t_pool.tile([P, GW], FP16, tag="t")
                s_t = s_pool.tile([P, GW], FP16, tag="s")
                nc.sync.dma_start(out=t_t[:, :gw], in_=t_in[g, :, :gw])
                nc.sync.dma_start(out=s_t[:, :gw], in_=s_in[g, :, :gw])

                # d = t - s first (reads raw t, s), then exp in place.
                d_t = d_pool.tile([P, GW], FP16, tag="d")
                nc.vector.tensor_tensor(
                    out=d_t[:, :gw], in0=t_t[:, :gw], in1=s_t[:, :gw], op=SUB
                )
                # et overwrites t, es overwrites s
                nc.scalar.activation(t_t[:, :gw], t_t[:, :gw], EXP, scale=1.0 / TEMP)
                nc.scalar.activation(s_t[:, :gw], s_t[:, :gw], EXP, scale=1.0 / TEMP)
                et_t, es_t = t_t, s_t

                sq_t = sq_pool.tile([P, GW], FP16, tag="sq")
                nc.vector.tensor_tensor(
                    out=sq_t[:, :gw], in0=es_t[:, :gw], in1=es_t[:, :gw], op=MULT
                )
                b_t = b_pool.tile([P, GW], FP16, tag="b")
                nc.vector.tensor_tensor(
                    out=b_t[:, :gw], in0=sq_t[:, :gw], in1=sq_t[:, :gw], op=MULT
                )

                for c in range(gvc):
                    first = g == 0 and c == 0
                    last = g == N_G - 1 and c == gvc - 1
                    base = c * R
                    # W-diag: 4 row-blocks, stationary = et block
                    for rb in range(RB):
                        lo = base + rb * P
                        nc.tensor.matmul(
                            ps_w[:, rb * P : (rb + 1) * P],
                            et_t[:, lo : lo + P],
                            d_t[:, lo : lo + P],
                            start=first,
                            stop=last,
                        )
                    # A/C/B ones-stationary column reductions, concurrent on
                    # PE column groups 0/1/2.
                    nc.tensor.matmul(
                        ps_acb[0:1, :], ones[:], es_t[:, base : base + R],
                        start=first, stop=last, tile_position=(0, 0),
                    )
                    nc.tensor.matmul(
                        ps_acb[32:33, :], ones[:], et_t[:, base : base + R],
                        start=first, stop=last, tile_position=(0, 32),
                    )
                    nc.tensor.matmul(
                        ps_acb[64:65, :], ones[:], b_t[:, base : base + R],
                        start=first, stop=last, tile_position=(0, 64),
                    )

            sb_w = const_pool.tile([P, R], FP32, tag="sb_w")
            sb_acb = const_pool.tile([65, R], FP32, tag="sb_acb")
            nc.vector.tensor_copy(sb_w[:], ps_w[:])
            nc.vector.tensor_copy(sb_acb[:], ps_acb[:])
            nc.sync.dma_start(out=o_w[:, :], in_=sb_w[:])
            nc.sync.dma_start(out=o_acb[:, :], in_=sb_acb[:])
    return nc


_PROGRAM = None


def _get_program():
    global _PROGRAM
    if _PROGRAM is None:
        _PROGRAM = build_program()
    return _PROGRAM


def _to_vm_tiles(x16_core):
    """[R, V] fp16 (row-major core slice) -> [N_G, 128, GW] vocab-major
    (last group zero-padded)."""
    xt = np.ascontiguousarray(x16_core.T)               # [V, R]
    v = xt.reshape(N_VCH, P, R)                         # [vc, 128, R]
    out = np.zeros((N_G, P, GW), dtype=np.float16)
    vc0 = 0
    for g, gvc in enumerate(GROUP_SIZES):
        blk = v[vc0 : vc0 + gvc].transpose(1, 0, 2)     # [128, gvc, R]
        out[g, :, : gvc * R] = blk.reshape(P, gvc * R)
        vc0 += gvc
    return out


def combine_partials(W, A, C, Bq, s_label, valid):
    """Host-side (float64) reduction of per-row device partials."""
    W = W.astype(np.float64)
    A = A.astype(np.float64)
    C = C.astype(np.float64)
    Bq = Bq.astype(np.float64)

    kl = W / (TEMP * C) + np.log(A) - np.log(C)
    distill = (TEMP**2) * kl.sum() / kl.shape[0]

    nll = np.log(Bq) - s_label.astype(np.float64)
    valid = valid.astype(np.float64)
    task = (nll * valid).sum() / max(valid.sum(), 1.0)

    total = ALPHA * distill + (1.0 - ALPHA) * task
    return (
        np.float32(total),
        np.float32(distill),
        np.float32(task),
    )


def kernel(student_logits, teacher_logits, labels):
    global LAST_RESULT
    s32 = np.ascontiguousarray(np.asarray(student_logits, dtype=np.float32)).reshape(
        N, V
    )
    s16 = s32.astype(np.float16)
    t16 = (
        np.ascontiguousarray(np.asarray(teacher_logits, dtype=np.float32))
        .reshape(N, V)
        .astype(np.float16)
    )
    lab = np.asarray(labels).reshape(N).astype(np.int64)

    nc = _get_program()
    in_maps = [
        {
            "t_vm": _to_vm_tiles(t16[i * R : (i + 1) * R]),
            "s_vm": _to_vm_tiles(s16[i * R : (i + 1) * R]),
        }
        for i in range(N_CORES)
    ]
    res = run_bass_kernel_spmd(nc, in_maps, list(range(N_CORES)), trace=TRACE)
    LAST_RESULT = res

    # Per-core partials -> flattened row order (core -> row-block -> lane)
    Ws, As, Cs, Bs = [], [], [], []
    for r in res.results:
        ow = r["o_w"].reshape(P, RB, P)
        # diag: W[rb*128 + j] = ow[j, rb, j]
        Wd = np.einsum("jrj->rj", ow).reshape(R)
        Ws.append(Wd)
        acb = r["o_acb"]
        As.append(acb[0])
        Cs.append(acb[32])
        Bs.append(acb[64])
    W = np.concatenate(Ws)
    A = np.concatenate(As)
    C = np.concatenate(Cs)
    Bq = np.concatenate(Bs)

    s_label = s32[np.arange(N), lab]
    valid = lab != IGNORE_INDEX
    return combine_partials(W, A, C, Bq, s_label, valid)


# revision 13
# speedup vs baseline: 1.3878x; 1.3878x over previous
"""Distillation loss (KL + CE) kernel for Trainium2, 8 NeuronCores.

v2 strategy (vocab-major / transposed layout, PE-based reductions):
  - Flatten logits to [N=4096, V=32000]; shard 512 rows per core; cast to
    fp16 on the host.  Host also TRANSPOSES each core's slice to
    vocab-major [V, 512] and retiles it to [25 groups, 128 vocab
    partitions, 10 vocab-chunks x 512 rows] so each SBUF tile is one
    contiguous 1.31 MB DMA.
  - Per core the engines split the work (per-pass numbers, 16.4M elems):
      ACT (the hard floor, ~220us): es = exp(s/4), et = exp(t/4).
      DVE (3 fp16 2x TT passes, ~205us): d = t - s, sq = es*es, b = sq*sq.
      PE  (previously idle): all reductions, accumulated in fp32 PSUM
          across the full vocab:
            W-diag:  out[m,n] += sum_v et[v,m] * d[v,n]   per 128-row
                     block (the diagonal is sum_v et*d = the KL cross
                     term; host extracts it).  Stationary = et block.
            A/C/B:   ones-stationary column reductions: out[1, 512] +=
                     sum_v x[v, r] for x in {es, et, b}.
  - No max-subtraction (randn inputs; exp stays in fp16/fp32 range).
  - Host (float64) combine:
       KL_row  = W / (T*C) + ln A - ln C
       distill = T^2 * mean(KL_row)
       nll_row = ln B - s[row, label]   (label gather on host, f32 exact)
       task    = sum(nll*valid) / max(sum(valid), 1), valid = label != 0
       total   = alpha*distill + (1-alpha)*task
"""

import ml_dtypes
import numpy as np

import concourse.bass as bass
import concourse.mybir as mybir
from concourse import tile
from concourse.bass_utils import run_bass_kernel_spmd
from concourse.vector_clock import ScopedClock, VectorClock


# ---------------------------------------------------------------------------
# Workaround: the walrus build in this image rejects instructions that carry
# more than one sync wait ("Too many sync wait commands", setupSyncWait).
# Tile freely assigns several waits to one instruction.  Two patches:
#   1. _lower_ordered_insts: before lowering, hoist excess waits from every
#      scheduled instruction onto same-engine NoOps inserted just before it.
#   2. _drain_and_barrier: the kernel-tail drain gets the whole global
#      vector clock on one instruction; emit one drain per logical proc.
# ---------------------------------------------------------------------------
_MAX_WAITS = 1


def _split_inst_waits(nc, ordered):
    for bb_name, insts in ordered.items():
        out = []
        for inst in insts:
            si = inst.sync_info
            if si is not None and si.on_wait and len(si.on_wait) > _MAX_WAITS:
                waits = list(si.on_wait)
                excess, keep = waits[:-_MAX_WAITS], waits[-_MAX_WAITS:]
                for i in range(0, len(excess), _MAX_WAITS):
                    nop = mybir.InstNoOp(
                        name=nc.get_next_instruction_name(),
                        engine=inst.engine,
                        sync_info=mybir.SyncInfo(
                            on_wait=excess[i : i + _MAX_WAITS], on_update=[]
                        ),
                    )
                    out.append(nop)
                inst.sync_info = mybir.SyncInfo(
                    on_wait=keep, on_update=list(si.on_update)
                )
            out.append(inst)
        ordered[bb_name] = out


_orig_lower_ordered_insts = tile.TileContext._lower_ordered_insts


def _patched_lower_ordered_insts(self, ordered):
    _split_inst_waits(self.nc, ordered)
    return _orig_lower_ordered_insts(self, ordered)


def _split_drain_and_barrier(self, tick_clock, wait_clock):
    nc = self.nc
    gc = tick_clock.global_clock
    n = len(gc)
    for p in range(n):
        t = gc[p]
        if t <= 0:
            continue
        vec = [0] * n
        vec[p] = t
        di = nc.sync.drain()
        wait_clock.add_sem_waits(di.ins, ScopedClock({None: VectorClock(vec)}))
    nc.all_engine_barrier()
    assert self.sems is not None
    popped = nc._tile_sem_poison_stack.pop()
    assert popped is self._sem_poison
    nc.clear_and_free_semaphores(list(self.sems.allocated().values()))
    nc.all_engine_barrier()


if not getattr(tile.TileContext, "_dloss_patched", False):
    tile.TileContext._lower_ordered_insts = _patched_lower_ordered_insts
    tile.TileContext._drain_and_barrier = _split_drain_and_barrier
    tile.TileContext._dloss_patched = True

# ---------------------------------------------------------------------------

# Problem constants (hardcoded per spec nn_DistillationLoss_52982716564146)
B, S, V = 4, 1024, 32000
N = B * S                      # 4096 rows
N_CORES = 8
R = N // N_CORES               # 512 rows per core
P = 128                        # SBUF partitions
RB = R // P                    # 4 row-blocks per core
VCH = P                        # vocab per chunk (partition dim)
N_VCH = V // VCH               # 250 vocab-chunks
G_VCH = 14                     # vocab-chunks per group
GROUP_SIZES = [G_VCH] * (N_VCH // G_VCH) + (
    [N_VCH % G_VCH] if N_VCH % G_VCH else []
)                              # 17 x 14 + 1 x 12
N_G = len(GROUP_SIZES)
GW = G_VCH * R                 # max group tile free width = 7168
TEMP = 4.0
ALPHA = 0.7
IGNORE_INDEX = 0

FP32 = mybir.dt.float32
BF16 = mybir.dt.bfloat16
EXP = mybir.ActivationFunctionType.Exp
MULT = mybir.AluOpType.mult
SUB = mybir.AluOpType.subtract

TRACE = False
LAST_RESULT = None


def build_program():
    """Build the SPMD Bass program (identical on all cores).

    Inputs  (per core): t_vm, s_vm [N_G, 128, GW] fp16 (vocab-major tiles,
        last group zero-padded).
    Outputs (per core): o_w [128, R] f32   (diag holds W = sum et*(t-s)),
                        o_acb [65, R] f32 (rows 0/32/64 = A, C, B sums).

    The exp activations run in place (et overwrites t, es overwrites s) so
    larger group tiles fit in SBUF (fewer ACT instructions, less per-
    instruction overhead).  The A/C/B ones-stationary reductions are issued
    to distinct 32-column PE groups (tile_position (0,0)/(0,32)/(0,64)) so
    the hardware runs all three concurrently on separate XBUSes.
    """
    nc = bass.Bass(
        "TRN2",
        target_bir_lowering=False,
        debug=False,
        num_devices=N_CORES,
    )
    t_in = nc.dram_tensor("t_vm", [N_G, P, GW], BF16, kind="ExternalInput")
    s_in = nc.dram_tensor("s_vm", [N_G, P, GW], BF16, kind="ExternalInput")
    o_w = nc.dram_tensor("o_w", [P, R], FP32, kind="ExternalOutput")
    o_acb = nc.dram_tensor("o_acb", [65, R], FP32, kind="ExternalOutput")

    def emit_b_reduce(b_tile, gb, gvc_b):
        """ones-reduction of b (data group gb) into ps_acb row 64."""
        for c in range(gvc_b):
            nc.tensor.matmul(
                ps_acb[64:65, :],
                ones[:],
                b_tile[:, c * R : (c + 1) * R],
                start=(gb == 0 and c == 0),
                stop=(gb == N_G - 1 and c == gvc_b - 1),
                tile_position=(0, 64),
            )

    with tile.TileContext(nc) as tc:
        with (
            tc.tile_pool(name="t_pool", bufs=2) as t_pool,
            tc.tile_pool(name="s_pool", bufs=2) as s_pool,
            tc.tile_pool(name="et_pool", bufs=2) as et_pool,
            tc.tile_pool(name="d_pool", bufs=2) as d_pool,
            tc.tile_pool(name="sq_pool", bufs=2) as sq_pool,
            tc.tile_pool(name="b_pool", bufs=2) as b_pool,
            tc.tile_pool(name="const", bufs=1) as const_pool,
            tc.tile_pool(name="psum", bufs=1, space="PSUM") as psum_pool,
        ):
            ones = const_pool.tile([P, 1], BF16, tag="ones")
            nc.gpsimd.memset(ones[:], 1.0)

            ps_w = psum_pool.tile([P, R], FP32, tag="ps_w")
            ps_acb = psum_pool.tile([65, R], FP32, tag="ps_acb")

            b_prev = None  # (tile, data-group idx, gvc) lagged one group
            for g, gvc in enumerate(GROUP_SIZES):
                gw = gvc * R
                t_t = t_pool.tile([P, GW], BF16, tag="t")
                s_t = s_pool.tile([P, GW], BF16, tag="s")
                nc.sync.dma_start(out=t_t[:, :gw], in_=t_in[g, :, :gw])
                nc.sync.dma_start(out=s_t[:, :gw], in_=s_in[g, :, :gw])

                # et in its own tile: ACT starts as soon as t lands.
                et_t = et_pool.tile([P, GW], BF16, tag="et")
                nc.scalar.activation(et_t[:, :gw], t_t[:, :gw], EXP, scale=1.0 / TEMP)
                # d = t - s (raw tiles)
                d_t = d_pool.tile([P, GW], BF16, tag="d")
                nc.vector.tensor_tensor(
                    out=d_t[:, :gw], in0=t_t[:, :gw], in1=s_t[:, :gw], op=SUB
                )
                # es overwrites s (WAR on d handled by Tile ordering)
                nc.scalar.activation(s_t[:, :gw], s_t[:, :gw], EXP, scale=1.0 / TEMP)
                es_t = s_t

                sq_t = sq_pool.tile([P, GW], BF16, tag="sq")
                nc.vector.tensor_tensor(
                    out=sq_t[:, :gw], in0=es_t[:, :gw], in1=es_t[:, :gw], op=MULT
                )
                b_t = b_pool.tile([P, GW], BF16, tag="b")
                nc.vector.tensor_tensor(
                    out=b_t[:, :gw], in0=sq_t[:, :gw], in1=sq_t[:, :gw], op=MULT
                )

                for c in range(gvc):
                    first = g == 0 and c == 0
                    last = g == N_G - 1 and c == gvc - 1
                    base = c * R
                    # W-diag: 4 row-blocks, stationary = et block
                    for rb in range(RB):
                        lo = base + rb * P
                        nc.tensor.matmul(
                            ps_w[:, rb * P : (rb + 1) * P],
                            et_t[:, lo : lo + P],
                            d_t[:, lo : lo + P],
                            start=first,
                            stop=last,
                        )
                    # A/C ones-stationary column reductions, concurrent on
                    # PE column groups 0/1.
                    nc.tensor.matmul(
                        ps_acb[0:1, :], ones[:], es_t[:, base : base + R],
                        start=first, stop=last, tile_position=(0, 0),
                    )
                    nc.tensor.matmul(
                        ps_acb[32:33, :], ones[:], et_t[:, base : base + R],
                        start=first, stop=last, tile_position=(0, 32),
                    )
                # B-reduce lagged one group so PE never waits on this
                # group's full DVE chain.
                if b_prev is not None:
                    emit_b_reduce(*b_prev)
                b_prev = (b_t, g, gvc)

            emit_b_reduce(*b_prev)

            sb_w = const_pool.tile([P, R], FP32, tag="sb_w")
            sb_acb = const_pool.tile([65, R], FP32, tag="sb_acb")
            nc.vector.tensor_copy(sb_w[:], ps_w[:])
            nc.vector.tensor_copy(sb_acb[:], ps_acb[:])
            nc.sync.dma_start(out=o_w[:, :], in_=sb_w[:])
            nc.sync.dma_start(out=o_acb[:, :], in_=sb_acb[:])
    return nc


_PROGRAM = None


def _get_program():
    global _PROGRAM
    if _PROGRAM is None:
        _PROGRAM = build_program()
    return _PROGRAM


def _to_vm_tiles(x16_core):
    """[R, V] bf16 (row-major core slice) -> [N_G, 128, GW] vocab-major
    (last group zero-padded)."""
    xt = np.ascontiguousarray(x16_core.T)               # [V, R]
    v = xt.reshape(N_VCH, P, R)                         # [vc, 128, R]
    out = np.zeros((N_G, P, GW), dtype=ml_dtypes.bfloat16)
    vc0 = 0
    for g, gvc in enumerate(GROUP_SIZES):
        blk = v[vc0 : vc0 + gvc].transpose(1, 0, 2)     # [128, gvc, R]
        out[g, :, : gvc * R] = blk.reshape(P, gvc * R)
        vc0 += gvc
    return out


def combine_partials(W, A, C, Bq, s_label, valid):
    """Host-side (float64) reduction of per-row device partials."""
    W = W.astype(np.float64)
    A = A.astype(np.float64)
    C = C.astype(np.float64)
    Bq = Bq.astype(np.float64)

    kl = W / (TEMP * C) + np.log(A) - np.log(C)
    distill = (TEMP**2) * kl.sum() / kl.shape[0]

    nll = np.log(Bq) - s_label.astype(np.float64)
    valid = valid.astype(np.float64)
    task = (nll * valid).sum() / max(valid.sum(), 1.0)

    total = ALPHA * distill + (1.0 - ALPHA) * task
    return (
        np.float32(total),
        np.float32(distill),
        np.float32(task),
    )


def kernel(student_logits, teacher_logits, labels):
    global LAST_RESULT
    s32 = np.ascontiguousarray(np.asarray(student_logits, dtype=np.float32)).reshape(
        N, V
    )
    s16 = s32.astype(ml_dtypes.bfloat16)
    t16 = (
        np.ascontiguousarray(np.asarray(teacher_logits, dtype=np.float32))
        .reshape(N, V)
        .astype(ml_dtypes.bfloat16)
    )
    lab = np.asarray(labels).reshape(N).astype(np.int64)

    nc = _get_program()
    in_maps = [
        {
            "t_vm": _to_vm_tiles(t16[i * R : (i + 1) * R]),
            "s_vm": _to_vm_tiles(s16[i * R : (i + 1) * R]),
        }
        for i in range(N_CORES)
    ]
    res = run_bass_kernel_spmd(nc, in_maps, list(range(N_CORES)), trace=TRACE)
    LAST_RESULT = res

    # Per-core partials -> flattened row order (core -> row-block -> lane)
    Ws, As, Cs, Bs = [], [], [], []
    for r in res.results:
        ow = r["o_w"].reshape(P, RB, P)
        # diag: W[rb*128 + j] = ow[j, rb, j]
        Wd = np.einsum("jrj->rj", ow).reshape(R)
        Ws.append(Wd)
        acb = r["o_acb"]
        As.append(acb[0])
        Cs.append(acb[32])
        Bs.append(acb[64])
    W = np.concatenate(Ws)
    A = np.concatenate(As)
    C = np.concatenate(Cs)
    Bq = np.concatenate(Bs)

    s_label = s32[np.arange(N), lab]
    valid = lab != IGNORE_INDEX
    return combine_partials(W, A, C, Bq, s_label, valid)


# revision 18
# speedup vs baseline: 1.3984x; 1.0076x over previous
"""Distillation loss (KL + CE) kernel for Trainium2, 8 NeuronCores.

v2 strategy (vocab-major / transposed layout, PE-based reductions):
  - Flatten logits to [N=4096, V=32000]; shard 512 rows per core; cast to
    fp16 on the host.  Host also TRANSPOSES each core's slice to
    vocab-major [V, 512] and retiles it to [25 groups, 128 vocab
    partitions, 10 vocab-chunks x 512 rows] so each SBUF tile is one
    contiguous 1.31 MB DMA.
  - Per core the engines split the work (per-pass numbers, 16.4M elems):
      ACT (the hard floor, ~220us): es = exp(s/4), et = exp(t/4).
      DVE (3 fp16 2x TT passes, ~205us): d = t - s, sq = es*es, b = sq*sq.
      PE  (previously idle): all reductions, accumulated in fp32 PSUM
          across the full vocab:
            W-diag:  out[m,n] += sum_v et[v,m] * d[v,n]   per 128-row
                     block (the diagonal is sum_v et*d = the KL cross
                     term; host extracts it).  Stationary = et block.
            A/C/B:   ones-stationary column reductions: out[1, 512] +=
                     sum_v x[v, r] for x in {es, et, b}.
  - No max-subtraction (randn inputs; exp stays in fp16/fp32 range).
  - Host (float64) combine:
       KL_row  = W / (T*C) + ln A - ln C
       distill = T^2 * mean(KL_row)
       nll_row = ln B - s[row, label]   (label gather on host, f32 exact)
       task    = sum(nll*valid) / max(sum(valid), 1), valid = label != 0
       total   = alpha*distill + (1-alpha)*task
"""

import ml_dtypes
import numpy as np

import concourse.bass as bass
import concourse.mybir as mybir
from concourse import tile
from concourse.bass_utils import run_bass_kernel_spmd
from concourse.vector_clock import ScopedClock, VectorClock


# ---------------------------------------------------------------------------
# Workaround: the walrus build in this image rejects instructions that carry
# more than one sync wait ("Too many sync wait commands", setupSyncWait).
# Tile freely assigns several waits to one instruction.  Two patches:
#   1. _lower_ordered_insts: before lowering, hoist excess waits from every
#      scheduled instruction onto same-engine NoOps inserted just before it.
#   2. _drain_and_barrier: the kernel-tail drain gets the whole global
#      vector clock on one instruction; emit one drain per logical proc.
# ---------------------------------------------------------------------------
_MAX_WAITS = 1


def _split_inst_waits(nc, ordered):
    for bb_name, insts in ordered.items():
        out = []
        for inst in insts:
            si = inst.sync_info
            if si is not None and si.on_wait and len(si.on_wait) > _MAX_WAITS:
                waits = list(si.on_wait)
                excess, keep = waits[:-_MAX_WAITS], waits[-_MAX_WAITS:]
                for i in range(0, len(excess), _MAX_WAITS):
                    nop = mybir.InstNoOp(
                        name=nc.get_next_instruction_name(),
                        engine=inst.engine,
                        sync_info=mybir.SyncInfo(
                            on_wait=excess[i : i + _MAX_WAITS], on_update=[]
                        ),
                    )
                    out.append(nop)
                inst.sync_info = mybir.SyncInfo(
                    on_wait=keep, on_update=list(si.on_update)
                )
            out.append(inst)
        ordered[bb_name] = out


_orig_lower_ordered_insts = tile.TileContext._lower_ordered_insts


def _patched_lower_ordered_insts(self, ordered):
    _split_inst_waits(self.nc, ordered)
    return _orig_lower_ordered_insts(self, ordered)


def _split_drain_and_barrier(self, tick_clock, wait_clock):
    nc = self.nc
    gc = tick_clock.global_clock
    n = len(gc)
    for p in range(n):
        t = gc[p]
        if t <= 0:
            continue
        vec = [0] * n
        vec[p] = t
        di = nc.sync.drain()
        wait_clock.add_sem_waits(di.ins, ScopedClock({None: VectorClock(vec)}))
    nc.all_engine_barrier()
    assert self.sems is not None
    popped = nc._tile_sem_poison_stack.pop()
    assert popped is self._sem_poison
    nc.clear_and_free_semaphores(list(self.sems.allocated().values()))
    nc.all_engine_barrier()


if not getattr(tile.TileContext, "_dloss_patched", False):
    tile.TileContext._lower_ordered_insts = _patched_lower_ordered_insts
    tile.TileContext._drain_and_barrier = _split_drain_and_barrier
    tile.TileContext._dloss_patched = True

# ---------------------------------------------------------------------------

# Problem constants (hardcoded per spec nn_DistillationLoss_52982716564146)
B, S, V = 4, 1024, 32000
N = B * S                      # 4096 rows
N_CORES = 8
R = N // N_CORES               # 512 rows per core
P = 128                        # SBUF partitions
RB = R // P                    # 4 row-blocks per core
VCH = P                        # vocab per chunk (partition dim)
N_VCH = V // VCH               # 250 vocab-chunks
G_VCH = 14                     # vocab-chunks per group
GROUP_SIZES = [G_VCH] * (N_VCH // G_VCH) + (
    [N_VCH % G_VCH] if N_VCH % G_VCH else []
)                              # 17 x 14 + 1 x 12
N_G = len(GROUP_SIZES)
GW = G_VCH * R                 # max group tile free width = 7168
TEMP = 4.0
ALPHA = 0.7
IGNORE_INDEX = 0

FP32 = mybir.dt.float32
BF16 = mybir.dt.bfloat16
EXP = mybir.ActivationFunctionType.Exp
MULT = mybir.AluOpType.mult
SUB = mybir.AluOpType.subtract

TRACE = False
LAST_RESULT = None


def build_program(with_acb=True, with_w=True):
    """Build the SPMD Bass program (identical on all cores).

    Inputs  (per core): t_vm, s_vm [N_G, 128, GW] fp16 (vocab-major tiles,
        last group zero-padded).
    Outputs (per core): o_w [128, R] f32   (diag holds W = sum et*(t-s)),
                        o_acb [65, R] f32 (rows 0/32/64 = A, C, B sums).

    The exp activations run in place (et overwrites t, es overwrites s) so
    larger group tiles fit in SBUF (fewer ACT instructions, less per-
    instruction overhead).  The A/C/B ones-stationary reductions are issued
    to distinct 32-column PE groups (tile_position (0,0)/(0,32)/(0,64)) so
    the hardware runs all three concurrently on separate XBUSes.
    """
    nc = bass.Bass(
        "TRN2",
        target_bir_lowering=False,
        debug=False,
        num_devices=N_CORES,
    )
    t_in = nc.dram_tensor("t_vm", [N_G, P, GW], BF16, kind="ExternalInput")
    s_in = nc.dram_tensor("s_vm", [N_G, P, GW], BF16, kind="ExternalInput")
    o_w = nc.dram_tensor("o_w", [P, R], FP32, kind="ExternalOutput")
    o_acb = nc.dram_tensor("o_acb", [65, R], FP32, kind="ExternalOutput")

    def emit_b_reduce(b_tile, gb, gvc_b):
        """ones-reduction of b (data group gb) into ps_acb row 64."""
        for c in range(gvc_b):
            nc.tensor.matmul(
                ps_acb[64:65, :],
                ones[:],
                b_tile[:, c * R : (c + 1) * R],
                start=False,
                stop=(gb == N_G - 1 and c == gvc_b - 1),
                tile_position=(0, 64),
                skip_group_check=True,
            )

    with tile.TileContext(nc) as tc:
        with (
            tc.tile_pool(name="t_pool", bufs=2) as t_pool,
            tc.tile_pool(name="s_pool", bufs=2) as s_pool,
            tc.tile_pool(name="et_pool", bufs=2) as et_pool,
            tc.tile_pool(name="d_pool", bufs=2) as d_pool,
            tc.tile_pool(name="sq_pool", bufs=2) as sq_pool,
            tc.tile_pool(name="b_pool", bufs=2) as b_pool,
            tc.tile_pool(name="const", bufs=1) as const_pool,
            tc.tile_pool(name="psum", bufs=1, space="PSUM") as psum_pool,
        ):
            ones = const_pool.tile([P, 1], BF16, tag="ones")
            nc.gpsimd.memset(ones[:], 1.0)

            ps_w = psum_pool.tile([P, R], FP32, tag="ps_w")
            ps_acb = psum_pool.tile([65, R], FP32, tag="ps_acb")
            # All regions of ps_w share one PSUM bank; a start=True matmul
            # clears has_written for the WHOLE bank partition-row, wiping
            # sibling regions' first contributions.  Instead: zero the
            # banks explicitly and accumulate with start=False throughout
            # (first write accumulates onto the memset zeros either way).
            nc.vector.memset(ps_w[:], 0.0)
            nc.vector.memset(ps_acb[:], 0.0)

            b_prev = None  # (tile, data-group idx, gvc) lagged one group
            for g, gvc in enumerate(GROUP_SIZES):
                gw = gvc * R
                t_t = t_pool.tile([P, GW], BF16, tag="t")
                s_t = s_pool.tile([P, GW], BF16, tag="s")
                nc.sync.dma_start(out=t_t[:, :gw], in_=t_in[g, :, :gw])
                nc.sync.dma_start(out=s_t[:, :gw], in_=s_in[g, :, :gw])

                # et in its own tile: ACT starts as soon as t lands.
                et_t = et_pool.tile([P, GW], BF16, tag="et")
                nc.scalar.activation(et_t[:, :gw], t_t[:, :gw], EXP, scale=1.0 / TEMP)
                # d = t - s (raw tiles)
                d_t = d_pool.tile([P, GW], BF16, tag="d")
                nc.vector.tensor_tensor(
                    out=d_t[:, :gw], in0=t_t[:, :gw], in1=s_t[:, :gw], op=SUB
                )
                # es overwrites s (WAR on d handled by Tile ordering)
                nc.scalar.activation(s_t[:, :gw], s_t[:, :gw], EXP, scale=1.0 / TEMP)
                es_t = s_t

                sq_t = sq_pool.tile([P, GW], BF16, tag="sq")
                nc.vector.tensor_tensor(
                    out=sq_t[:, :gw], in0=es_t[:, :gw], in1=es_t[:, :gw], op=MULT
                )
                b_t = b_pool.tile([P, GW], BF16, tag="b")
                nc.vector.tensor_tensor(
                    out=b_t[:, :gw], in0=sq_t[:, :gw], in1=sq_t[:, :gw], op=MULT
                )

                for c in range(gvc):
                    first = g == 0 and c == 0
                    last = g == N_G - 1 and c == gvc - 1
                    base = c * R
                    # W-diag: 4 row-blocks, stationary = et block
                    if with_w:
                        for rb in range(RB):
                            lo = base + rb * P
                            nc.tensor.matmul(
                                ps_w[:, rb * P : (rb + 1) * P],
                                et_t[:, lo : lo + P],
                                d_t[:, lo : lo + P],
                                start=False,
                                stop=last,
                                skip_group_check=True,
                            )
                    # A/C ones-stationary column reductions, concurrent on
                    # PE column groups 0/1.
                    if with_acb:
                        nc.tensor.matmul(
                            ps_acb[0:1, :], ones[:], es_t[:, base : base + R],
                            start=False, stop=last, tile_position=(0, 0),
                            skip_group_check=True,
                        )
                        nc.tensor.matmul(
                            ps_acb[32:33, :], ones[:], et_t[:, base : base + R],
                            start=False, stop=last, tile_position=(0, 32),
                            skip_group_check=True,
                        )
                # B-reduce lagged one group so PE never waits on this
                # group's full DVE chain.
                if with_acb:
                    if b_prev is not None:
                        emit_b_reduce(*b_prev)
                    b_prev = (b_t, g, gvc)

            if with_acb:
                emit_b_reduce(*b_prev)

            sb_w = const_pool.tile([P, R], FP32, tag="sb_w")
            sb_acb = const_pool.tile([65, R], FP32, tag="sb_acb")
            if with_w:
                nc.vector.tensor_copy(sb_w[:], ps_w[:])
            else:
                nc.vector.memset(sb_w[:], 0.0)
            if with_acb:
                nc.vector.tensor_copy(sb_acb[:], ps_acb[:])
            else:
                nc.vector.memset(sb_acb[:], 0.0)
            nc.sync.dma_start(out=o_w[:, :], in_=sb_w[:])
            nc.sync.dma_start(out=o_acb[:, :], in_=sb_acb[:])
    return nc


_PROGRAM = None


def _get_program():
    global _PROGRAM
    if _PROGRAM is None:
        _PROGRAM = build_program()
    return _PROGRAM


def _to_vm_tiles(x16_core):
    """[R, V] bf16 (row-major core slice) -> [N_G, 128, GW] vocab-major
    (last group zero-padded)."""
    xt = np.ascontiguousarray(x16_core.T)               # [V, R]
    v = xt.reshape(N_VCH, P, R)                         # [vc, 128, R]
    out = np.zeros((N_G, P, GW), dtype=ml_dtypes.bfloat16)
    vc0 = 0
    for g, gvc in enumerate(GROUP_SIZES):
        blk = v[vc0 : vc0 + gvc].transpose(1, 0, 2)     # [128, gvc, R]
        out[g, :, : gvc * R] = blk.reshape(P, gvc * R)
        vc0 += gvc
    return out


def combine_partials(W, A, C, Bq, s_label, valid):
    """Host-side (float64) reduction of per-row device partials."""
    W = W.astype(np.float64)
    A = A.astype(np.float64)
    C = C.astype(np.float64)
    Bq = Bq.astype(np.float64)

    kl = W / (TEMP * C) + np.log(A) - np.log(C)
    distill = (TEMP**2) * kl.sum() / kl.shape[0]

    nll = np.log(Bq) - s_label.astype(np.float64)
    valid = valid.astype(np.float64)
    task = (nll * valid).sum() / max(valid.sum(), 1.0)

    total = ALPHA * distill + (1.0 - ALPHA) * task
    return (
        np.float32(total),
        np.float32(distill),
        np.float32(task),
    )


def kernel(student_logits, teacher_logits, labels):
    global LAST_RESULT
    s32 = np.ascontiguousarray(np.asarray(student_logits, dtype=np.float32)).reshape(
        N, V
    )
    s16 = s32.astype(ml_dtypes.bfloat16)
    t16 = (
        np.ascontiguousarray(np.asarray(teacher_logits, dtype=np.float32))
        .reshape(N, V)
        .astype(ml_dtypes.bfloat16)
    )
    lab = np.asarray(labels).reshape(N).astype(np.int64)

    nc = _get_program()
    in_maps = [
        {
            "t_vm": _to_vm_tiles(t16[i * R : (i + 1) * R]),
            "s_vm": _to_vm_tiles(s16[i * R : (i + 1) * R]),
        }
        for i in range(N_CORES)
    ]
    res = run_bass_kernel_spmd(nc, in_maps, list(range(N_CORES)), trace=TRACE)
    LAST_RESULT = res

    # Per-core partials -> flattened row order (core -> row-block -> lane)
    Ws, As, Cs, Bs = [], [], [], []
    for r in res.results:
        ow = r["o_w"].reshape(P, RB, P)
        # diag: W[rb*128 + j] = ow[j, rb, j]
        Wd = np.einsum("jrj->rj", ow).reshape(R)
        Ws.append(Wd)
        acb = r["o_acb"]
        As.append(acb[0])
        Cs.append(acb[32])
        Bs.append(acb[64])
    W = np.concatenate(Ws)
    A = np.concatenate(As)
    C = np.concatenate(Cs)
    Bq = np.concatenate(Bs)

    s_label = s32[np.arange(N), lab]
    valid = lab != IGNORE_INDEX
    return combine_partials(W, A, C, Bq, s_label, valid)


# revision 20
# speedup vs baseline: 1.5263x; 1.0915x over previous
"""Distillation loss (KL + CE) kernel for Trainium2, 8 NeuronCores.

v2 strategy (vocab-major / transposed layout, PE-based reductions):
  - Flatten logits to [N=4096, V=32000]; shard 512 rows per core; cast to
    fp16 on the host.  Host also TRANSPOSES each core's slice to
    vocab-major [V, 512] and retiles it to [25 groups, 128 vocab
    partitions, 10 vocab-chunks x 512 rows] so each SBUF tile is one
    contiguous 1.31 MB DMA.
  - Per core the engines split the work (per-pass numbers, 16.4M elems):
      ACT (the hard floor, ~220us): es = exp(s/4), et = exp(t/4).
      DVE (3 fp16 2x TT passes, ~205us): d = t - s, sq = es*es, b = sq*sq.
      PE  (previously idle): all reductions, accumulated in fp32 PSUM
          across the full vocab:
            W-diag:  out[m,n] += sum_v et[v,m] * d[v,n]   per 128-row
                     block (the diagonal is sum_v et*d = the KL cross
                     term; host extracts it).  Stationary = et block.
            A/C/B:   ones-stationary column reductions: out[1, 512] +=
                     sum_v x[v, r] for x in {es, et, b}.
  - No max-subtraction (randn inputs; exp stays in fp16/fp32 range).
  - Host (float64) combine:
       KL_row  = W / (T*C) + ln A - ln C
       distill = T^2 * mean(KL_row)
       nll_row = ln B - s[row, label]   (label gather on host, f32 exact)
       task    = sum(nll*valid) / max(sum(valid), 1), valid = label != 0
       total   = alpha*distill + (1-alpha)*task
"""

import ml_dtypes
import numpy as np

import concourse.bass as bass
import concourse.mybir as mybir
from concourse import tile
from concourse.bass_utils import run_bass_kernel_spmd
from concourse.vector_clock import ScopedClock, VectorClock


# ---------------------------------------------------------------------------
# Workaround: the walrus build in this image rejects instructions that carry
# more than one sync wait ("Too many sync wait commands", setupSyncWait).
# Tile freely assigns several waits to one instruction.  Two patches:
#   1. _lower_ordered_insts: before lowering, hoist excess waits from every
#      scheduled instruction onto same-engine NoOps inserted just before it.
#   2. _drain_and_barrier: the kernel-tail drain gets the whole global
#      vector clock on one instruction; emit one drain per logical proc.
# ---------------------------------------------------------------------------
_MAX_WAITS = 1


def _split_inst_waits(nc, ordered):
    for bb_name, insts in ordered.items():
        out = []
        for inst in insts:
            si = inst.sync_info
            if si is not None and si.on_wait and len(si.on_wait) > _MAX_WAITS:
                waits = list(si.on_wait)
                excess, keep = waits[:-_MAX_WAITS], waits[-_MAX_WAITS:]
                for i in range(0, len(excess), _MAX_WAITS):
                    nop = mybir.InstNoOp(
                        name=nc.get_next_instruction_name(),
                        engine=inst.engine,
                        sync_info=mybir.SyncInfo(
                            on_wait=excess[i : i + _MAX_WAITS], on_update=[]
                        ),
                    )
                    out.append(nop)
                inst.sync_info = mybir.SyncInfo(
                    on_wait=keep, on_update=list(si.on_update)
                )
            out.append(inst)
        ordered[bb_name] = out


_orig_lower_ordered_insts = tile.TileContext._lower_ordered_insts


def _patched_lower_ordered_insts(self, ordered):
    _split_inst_waits(self.nc, ordered)
    return _orig_lower_ordered_insts(self, ordered)


def _split_drain_and_barrier(self, tick_clock, wait_clock):
    nc = self.nc
    gc = tick_clock.global_clock
    n = len(gc)
    for p in range(n):
        t = gc[p]
        if t <= 0:
            continue
        vec = [0] * n
        vec[p] = t
        di = nc.sync.drain()
        wait_clock.add_sem_waits(di.ins, ScopedClock({None: VectorClock(vec)}))
    nc.all_engine_barrier()
    assert self.sems is not None
    popped = nc._tile_sem_poison_stack.pop()
    assert popped is self._sem_poison
    nc.clear_and_free_semaphores(list(self.sems.allocated().values()))
    nc.all_engine_barrier()


if not getattr(tile.TileContext, "_dloss_patched", False):
    tile.TileContext._lower_ordered_insts = _patched_lower_ordered_insts
    tile.TileContext._drain_and_barrier = _split_drain_and_barrier
    tile.TileContext._dloss_patched = True

# ---------------------------------------------------------------------------

# Problem constants (hardcoded per spec nn_DistillationLoss_52982716564146)
B, S, V = 4, 1024, 32000
N = B * S                      # 4096 rows
N_CORES = 8
R = N // N_CORES               # 512 rows per core
P = 128                        # SBUF partitions
RB = R // P                    # 4 row-blocks per core
VCH = P                        # vocab per chunk (partition dim)
N_VCH = V // VCH               # 250 vocab-chunks
G_VCH = 12                     # vocab-chunks per (main) group
# Small first group so ACT starts early; small last group so the tail
# (last DVE chain + lagged B reduction + output copy) is short.
GROUP_SIZES = [4] + [G_VCH] * 20 + [6]   # 4 + 240 + 6 = 250
assert sum(GROUP_SIZES) == N_VCH
N_G = len(GROUP_SIZES)
GW = G_VCH * R                 # max group tile free width = 6144
TEMP = 4.0
ALPHA = 0.7
IGNORE_INDEX = 0

FP32 = mybir.dt.float32
BF16 = mybir.dt.bfloat16
EXP = mybir.ActivationFunctionType.Exp
MULT = mybir.AluOpType.mult
SUB = mybir.AluOpType.subtract

TRACE = False
LAST_RESULT = None


def build_program(with_acb=True, with_w=True):
    """Build the SPMD Bass program (identical on all cores).

    Inputs  (per core): t_vm, s_vm [N_G, 128, GW] fp16 (vocab-major tiles,
        last group zero-padded).
    Outputs (per core): o_w [128, R] f32   (diag holds W = sum et*(t-s)),
                        o_acb [65, R] f32 (rows 0/32/64 = A, C, B sums).

    The exp activations run in place (et overwrites t, es overwrites s) so
    larger group tiles fit in SBUF (fewer ACT instructions, less per-
    instruction overhead).  The A/C/B ones-stationary reductions are issued
    to distinct 32-column PE groups (tile_position (0,0)/(0,32)/(0,64)) so
    the hardware runs all three concurrently on separate XBUSes.
    """
    nc = bass.Bass(
        "TRN2",
        target_bir_lowering=False,
        debug=False,
        num_devices=N_CORES,
    )
    t_in = nc.dram_tensor("t_vm", [N_G, P, GW], BF16, kind="ExternalInput")
    s_in = nc.dram_tensor("s_vm", [N_G, P, GW], BF16, kind="ExternalInput")
    o_w = nc.dram_tensor("o_w", [P, R], FP32, kind="ExternalOutput")
    o_acb = nc.dram_tensor("o_acb", [65, R], FP32, kind="ExternalOutput")

    def emit_b_reduce(b_tile, gb, gvc_b):
        """ones-reduction of b (data group gb) into ps_acb row 64."""
        for c in range(gvc_b):
            nc.tensor.matmul(
                ps_acb[64:65, :],
                ones[:],
                b_tile[:, c * R : (c + 1) * R],
                start=False,
                stop=(gb == N_G - 1 and c == gvc_b - 1),
                tile_position=(0, 64),
                skip_group_check=True,
            )

    with tile.TileContext(nc) as tc:
        with (
            tc.tile_pool(name="t_pool", bufs=3) as t_pool,
            tc.tile_pool(name="s_pool", bufs=3) as s_pool,
            tc.tile_pool(name="et_pool", bufs=2) as et_pool,
            tc.tile_pool(name="d_pool", bufs=2) as d_pool,
            tc.tile_pool(name="sq_pool", bufs=2) as sq_pool,
            tc.tile_pool(name="b_pool", bufs=2) as b_pool,
            tc.tile_pool(name="const", bufs=1) as const_pool,
            tc.tile_pool(name="psum", bufs=1, space="PSUM") as psum_pool,
        ):
            ones = const_pool.tile([P, 1], BF16, tag="ones")
            nc.gpsimd.memset(ones[:], 1.0)

            ps_w = psum_pool.tile([P, R], FP32, tag="ps_w")
            ps_acb = psum_pool.tile([65, R], FP32, tag="ps_acb")
            # All regions of ps_w share one PSUM bank; a start=True matmul
            # clears has_written for the WHOLE bank partition-row, wiping
            # sibling regions' first contributions.  Instead: zero the
            # banks explicitly and accumulate with start=False throughout
            # (first write accumulates onto the memset zeros either way).
            nc.vector.memset(ps_w[:], 0.0)
            nc.vector.memset(ps_acb[:], 0.0)

            b_prev = None  # (tile, data-group idx, gvc) lagged one group
            for g, gvc in enumerate(GROUP_SIZES):
                gw = gvc * R
                t_t = t_pool.tile([P, GW], BF16, tag="t")
                s_t = s_pool.tile([P, GW], BF16, tag="s")
                nc.sync.dma_start(out=t_t[:, :gw], in_=t_in[g, :, :gw])
                nc.sync.dma_start(out=s_t[:, :gw], in_=s_in[g, :, :gw])

                # et in its own tile: ACT starts as soon as t lands.
                et_t = et_pool.tile([P, GW], BF16, tag="et")
                nc.scalar.activation(et_t[:, :gw], t_t[:, :gw], EXP, scale=1.0 / TEMP)
                # d = t - s (raw tiles)
                d_t = d_pool.tile([P, GW], BF16, tag="d")
                nc.vector.tensor_tensor(
                    out=d_t[:, :gw], in0=t_t[:, :gw], in1=s_t[:, :gw], op=SUB
                )
                # es overwrites s (WAR on d handled by Tile ordering)
                nc.scalar.activation(s_t[:, :gw], s_t[:, :gw], EXP, scale=1.0 / TEMP)
                es_t = s_t

                sq_t = sq_pool.tile([P, GW], BF16, tag="sq")
                nc.vector.tensor_tensor(
                    out=sq_t[:, :gw], in0=es_t[:, :gw], in1=es_t[:, :gw], op=MULT
                )
                b_t = b_pool.tile([P, GW], BF16, tag="b")
                nc.vector.tensor_tensor(
                    out=b_t[:, :gw], in0=sq_t[:, :gw], in1=sq_t[:, :gw], op=MULT
                )

                for c in range(gvc):
                    first = g == 0 and c == 0
                    last = g == N_G - 1 and c == gvc - 1
                    base = c * R
                    # W-diag: 4 row-blocks, stationary = et block
                    if with_w:
                        for rb in range(RB):
                            lo = base + rb * P
                            nc.tensor.matmul(
                                ps_w[:, rb * P : (rb + 1) * P],
                                et_t[:, lo : lo + P],
                                d_t[:, lo : lo + P],
                                start=False,
                                stop=last,
                                skip_group_check=True,
                            )
                    # A/C ones-stationary column reductions, concurrent on
                    # PE column groups 0/1.
                    if with_acb:
                        nc.tensor.matmul(
                            ps_acb[0:1, :], ones[:], es_t[:, base : base + R],
                            start=False, stop=last, tile_position=(0, 0),
                            skip_group_check=True,
                        )
                        nc.tensor.matmul(
                            ps_acb[32:33, :], ones[:], et_t[:, base : base + R],
                            start=False, stop=last, tile_position=(0, 32),
                            skip_group_check=True,
                        )
                # B-reduce lagged one group so PE never waits on this
                # group's full DVE chain.
                if with_acb:
                    if b_prev is not None:
                        emit_b_reduce(*b_prev)
                    b_prev = (b_t, g, gvc)

            if with_acb:
                emit_b_reduce(*b_prev)

            sb_w = const_pool.tile([P, R], FP32, tag="sb_w")
            sb_acb = const_pool.tile([65, R], FP32, tag="sb_acb")
            if with_w:
                nc.vector.tensor_copy(sb_w[:], ps_w[:])
            else:
                nc.vector.memset(sb_w[:], 0.0)
            if with_acb:
                nc.vector.tensor_copy(sb_acb[:], ps_acb[:])
            else:
                nc.vector.memset(sb_acb[:], 0.0)
            nc.sync.dma_start(out=o_w[:, :], in_=sb_w[:])
            nc.sync.dma_start(out=o_acb[:, :], in_=sb_acb[:])
    return nc


_PROGRAM = None


def _get_program():
    global _PROGRAM
    if _PROGRAM is None:
        _PROGRAM = build_program()
    return _PROGRAM


def _to_vm_tiles(x16_core):
    """[R, V] bf16 (row-major core slice) -> [N_G, 128, GW] vocab-major
    (last group zero-padded)."""
    xt = np.ascontiguousarray(x16_core.T)               # [V, R]
    v = xt.reshape(N_VCH, P, R)                         # [vc, 128, R]
    out = np.zeros((N_G, P, GW), dtype=ml_dtypes.bfloat16)
    vc0 = 0
    for g, gvc in enumerate(GROUP_SIZES):
        blk = v[vc0 : vc0 + gvc].transpose(1, 0, 2)     # [128, gvc, R]
        out[g, :, : gvc * R] = blk.reshape(P, gvc * R)
        vc0 += gvc
    return out


def combine_partials(W, A, C, Bq, s_label, valid):
    """Host-side (float64) reduction of per-row device partials."""
    W = W.astype(np.float64)
    A = A.astype(np.float64)
    C = C.astype(np.float64)
    Bq = Bq.astype(np.float64)

    kl = W / (TEMP * C) + np.log(A) - np.log(C)
    distill = (TEMP**2) * kl.sum() / kl.shape[0]

    nll = np.log(Bq) - s_label.astype(np.float64)
    valid = valid.astype(np.float64)
    task = (nll * valid).sum() / max(valid.sum(), 1.0)

    total = ALPHA * distill + (1.0 - ALPHA) * task
    return (
        np.float32(total),
        np.float32(distill),
        np.float32(task),
    )


def kernel(student_logits, teacher_logits, labels):
    global LAST_RESULT
    s32 = np.ascontiguousarray(np.asarray(student_logits, dtype=np.float32)).reshape(
        N, V
    )
    s16 = s32.astype(ml_dtypes.bfloat16)
    t16 = (
        np.ascontiguousarray(np.asarray(teacher_logits, dtype=np.float32))
        .reshape(N, V)
        .astype(ml_dtypes.bfloat16)
    )
    lab = np.asarray(labels).reshape(N).astype(np.int64)

    nc = _get_program()
    in_maps = [
        {
            "t_vm": _to_vm_tiles(t16[i * R : (i + 1) * R]),
            "s_vm": _to_vm_tiles(s16[i * R : (i + 1) * R]),
        }
        for i in range(N_CORES)
    ]
    res = run_bass_kernel_spmd(nc, in_maps, list(range(N_CORES)), trace=TRACE)
    LAST_RESULT = res

    # Per-core partials -> flattened row order (core -> row-block -> lane)
    Ws, As, Cs, Bs = [], [], [], []
    for r in res.results:
        ow = r["o_w"].reshape(P, RB, P)
        # diag: W[rb*128 + j] = ow[j, rb, j]
        Wd = np.einsum("jrj->rj", ow).reshape(R)
        Ws.append(Wd)
        acb = r["o_acb"]
        As.append(acb[0])
        Cs.append(acb[32])
        Bs.append(acb[64])
    W = np.concatenate(Ws)
    A = np.concatenate(As)
    C = np.concatenate(Cs)
    Bq = np.concatenate(Bs)

    s_label = s32[np.arange(N), lab]
    valid = lab != IGNORE_INDEX
    return combine_partials(W, A, C, Bq, s_label, valid)
